# revision 1
# baseline (speedup 1.0000x reference)
"""Multi-head causal attention (B=4,S=2048,D=768,H=12,HD=64) on 8 Trainium2 cores.

Sharding: 4-way head tensor-parallel (3 heads/core) x 2-way batch data-parallel
(2 batches/core).  Core c: batch group bg=c//4 (batches 2bg,2bg+1), head group
hg=c%4 (heads 3hg..3hg+2).

Per-core device program (SPMD, identical on all cores; differences come from data):
  1. q/k projections emitted transposed (qT,kT: [64 head-dim partitions, rows]),
     v projection row-major with an appended ones column (for the softmax
     denominator).
  2. Causal attention computed as S_T[k,q] = kT.T @ qT so that P=exp(S_T) feeds
     the A@V matmul directly as lhsT (no P transpose).  Softmax skips the
     running-max (scores are O(1) for this problem's scale).  Two batches of the
     same head run concurrently on the PE via 64-row tile packing
     (tile_position row groups).  Denominator l arrives as column 64 of the AV
     psum via the ones column of v; normalize is a per-partition scalar multiply.
  3. Per-head context transposed (PE transpose) into ctxT; output projection
     with the bias folded in as an extra contraction row (bp/4 so the 4-way
     sum reconstructs bp).
  4. Chunked ReduceScatter (groups {0..3},{4..7}) over the partial projection,
     overlapped with compute of later chunks.

Host side only slices/casts/transposes inputs and concatenates output shards.
"""

import sys

if "/opt/trn_rl_repo" not in sys.path:
    sys.path.insert(0, "/opt/trn_rl_repo")

import numpy as np
import ml_dtypes

BF16 = ml_dtypes.bfloat16

B, S, D = 4, 2048, 768
H, HD = 12, 64
N_CORES = 8
BL = 2          # batches per core
HL = 3          # heads per core
R = BL * S      # 4096 rows per core
KSUB = D // 128  # 6

_CACHE = {}


def _build_nc():
    import concourse.bass as bass  # noqa: F401
    import concourse.tile as tile
    from concourse import bacc, mybir
    from concourse.masks import make_identity

    f32 = mybir.dt.float32
    bf16 = mybir.dt.bfloat16
    EXP = mybir.ActivationFunctionType.Exp

    nc = bacc.Bacc("TRN2", target_bir_lowering=False, debug=False,
                   num_devices=N_CORES)

    xT_d = nc.dram_tensor("xT", [D, R], bf16, kind="ExternalInput").ap()
    wqk_d = nc.dram_tensor("wqk", [D, 2 * HL * HD], bf16, kind="ExternalInput").ap()
    wv_d = nc.dram_tensor("wv", [D, HL * HD], bf16, kind="ExternalInput").ap()
    wp_d = nc.dram_tensor("wp", [256, D], bf16, kind="ExternalInput").ap()
    mk_d = nc.dram_tensor("mk", [4, 128, 512], bf16, kind="ExternalInput").ap()
    out_d = nc.dram_tensor("out", [4, 256, D], f32, kind="ExternalOutput").ap()

    with tile.TileContext(nc) as tc:
        with tc.tile_pool(name="persist", bufs=1) as per, \
             tc.tile_pool(name="dram", bufs=1, space="DRAM") as dram:
            # ---- persistent SBUF tensors -------------------------------
            xT = per.tile([128, KSUB, R], bf16, tag="xT")
            # split the 6 MB load across k-subtiles so DMA queues spread
            for j in range(KSUB):
                nc.sync.dma_start(
                    xT[:, j, :],
                    xT_d.rearrange("(o p) r -> p o r", p=128)[:, j, :])
            wqk = per.tile([128, KSUB, 2 * HL * HD], bf16, tag="wqk")
            nc.sync.dma_start(wqk[:], wqk_d.rearrange("(o p) c -> p o c", p=128))
            wv = per.tile([128, KSUB, HL * HD], bf16, tag="wv")
            nc.sync.dma_start(wv[:], wv_d.rearrange("(o p) c -> p o c", p=128))
            wp = per.tile([128, 2, D], bf16, tag="wp")
            nc.sync.dma_start(wp[:], wp_d.rearrange("(o p) c -> p o c", p=128))
            masks = per.tile([128, 4, 512], bf16, tag="mk")
            nc.sync.dma_start(masks[:], mk_d.rearrange("o p c -> p o c"))
            ident = per.tile([128, 128], bf16, tag="ident")
            make_identity(nc, ident[:])

            # pair p = head p; partitions 0:64 = batch 0, 64:128 = batch 1
            qT = per.tile([128, HL, S], bf16, tag="qT")
            kT = per.tile([128, HL, S], bf16, tag="kT")
            # v row-tiles with ones col per head: [.., t, 65*h+64] == 1
            vE = per.tile([128, 2 * 16, HL * 65], bf16, tag="vE")
            nc.vector.memset(vE[:], 1.0)

            # ---- phase 1: projections ---------------------------------
            with tc.tile_pool(name="pj_ps", bufs=2, space="PSUM") as pj_ps:
                for b in range(BL):
                    for rc in range(4):  # 512-row chunks
                        r0 = b * S + rc * 512
                        for ct in range(3):  # qk col tiles of 128
                            ps = pj_ps.tile([128, 512], f32, tag="qkps")
                            for j in range(KSUB):
                                nc.tensor.matmul(
                                    ps[:],
                                    lhsT=wqk[:, j, ct * 128:(ct + 1) * 128],
                                    rhs=xT[:, j, r0:r0 + 512],
                                    start=(j == 0), stop=(j == KSUB - 1))
                            for half in range(2):
                                gid = 2 * ct + half
                                dest = qT if gid < 3 else kT
                                pair = gid % 3
                                nc.vector.tensor_copy(
                                    dest[b * 64:(b + 1) * 64, pair,
                                         rc * 512:(rc + 1) * 512],
                                    ps[half * 64:(half + 1) * 64, :])
                    for rt in range(16):  # v proj row tiles of 128
                        r0 = b * S + rt * 128
                        psv = pj_ps.tile([128, HL * HD], f32, tag="vps")
                        for j in range(KSUB):
                            nc.tensor.matmul(
                                psv[:],
                                lhsT=xT[:, j, r0:r0 + 128],
                                rhs=wv[:, j, :],
                                start=(j == 0), stop=(j == KSUB - 1))
                        for h in range(HL):
                            nc.vector.tensor_copy(
                                vE[:, b * 16 + rt, h * 65:h * 65 + 64],
                                psv[:, h * 64:(h + 1) * 64])

            # ---- phase 2: attention + out-proj + chunked RS ------------
            with tc.tile_pool(name="st_ps", bufs=3, space="PSUM") as st_ps, \
                 tc.tile_pool(name="av_ps", bufs=2, space="PSUM") as av_ps, \
                 tc.tile_pool(name="tr_ps", bufs=1, space="PSUM") as tr_ps, \
                 tc.tile_pool(name="op_ps", bufs=2, space="PSUM") as op_ps, \
                 tc.tile_pool(name="pt", bufs=34) as ptp, \
                 tc.tile_pool(name="sm", bufs=4) as sm, \
                 tc.tile_pool(name="ctxp", bufs=2) as ctxp, \
                 tc.tile_pool(name="outp", bufs=3) as outp:
                for qb in range(4):
                    n_k = 4 * (qb + 1)
                    q0 = qb * 512
                    ctxT = ctxp.tile([128, 2, 1024], bf16, tag="ctxT")
                    nc.vector.memset(ctxT[64:65, 1, :], 1.0)  # bias ones row
                    for pair in range(HL):
                        pts = [[None] * n_k for _ in range(2)]
                        for kt in range(n_k):
                            for u in range(2):
                                ps_st = st_ps.tile([128, 512], f32, tag="st")
                                nc.tensor.matmul(
                                    ps_st[:],
                                    lhsT=kT[u * 64:(u + 1) * 64, pair,
                                            kt * 128:(kt + 1) * 128],
                                    rhs=qT[u * 64:(u + 1) * 64, pair,
                                           q0:q0 + 512],
                                    start=True, stop=True)
                                pt_t = ptp.tile([128, 512], bf16, tag="pt")
                                nc.scalar.activation(pt_t[:], ps_st[:], EXP,
                                                     scale=float(HD) ** -0.5)
                                if kt >= n_k - 4:
                                    o = kt - (n_k - 4)
                                    nc.vector.tensor_mul(pt_t[:], pt_t[:],
                                                         masks[:, o, :])
                                pts[u][kt] = pt_t
                        for u in range(2):
                            for qt in range(4):
                                av = av_ps.tile([128, 65], f32, tag="av")
                                for kt in range(n_k):
                                    nc.tensor.matmul(
                                        av[:],
                                        lhsT=pts[u][kt][:, qt * 128:(qt + 1) * 128],
                                        rhs=vE[:, u * 16 + kt,
                                               pair * 65:(pair + 1) * 65],
                                        start=(kt == 0), stop=(kt == n_k - 1))
                                rec = sm.tile([128, 1], f32, tag="rec")
                                nc.vector.reciprocal(rec[:], av[:, 64:65])
                                cn = sm.tile([128, HD], bf16, tag="cn")
                                nc.vector.tensor_scalar_mul(cn[:], av[:, 0:HD],
                                                            rec[:])
                                tp = tr_ps.tile([HD, 128], bf16, tag="tp")
                                nc.tensor.transpose(tp[:], cn[:], ident[:])
                                ks, poff = (0, pair * 64) if pair < 2 else (1, 0)
                                cols = u * 512 + qt * 128
                                nc.vector.tensor_copy(
                                    ctxT[poff:poff + 64, ks, cols:cols + 128],
                                    tp[:])
                    # out-projection for this chunk (1024 rows)
                    part = dram.tile([1024, D], f32, tag=f"part{qb}")
                    for rt in range(8):
                        c0 = rt * 128
                        osb = outp.tile([128, D], f32, tag="osb")
                        for nh in range(2):
                            po = op_ps.tile([128, 384], f32, tag="op")
                            n0 = nh * 384
                            nc.tensor.matmul(po[:],
                                             lhsT=ctxT[:, 0, c0:c0 + 128],
                                             rhs=wp[:, 0, n0:n0 + 384],
                                             start=True, stop=False)
                            nc.tensor.matmul(po[:],
                                             lhsT=ctxT[0:65, 1, c0:c0 + 128],
                                             rhs=wp[0:65, 1, n0:n0 + 384],
                                             start=False, stop=True)
                            nc.vector.tensor_copy(osb[:, n0:n0 + 384], po[:])
                        nc.sync.dma_start(part[c0:c0 + 128, :], osb[:])
                    rso = dram.tile([256, D], f32, tag=f"rso{qb}")
                    nc.gpsimd.collective_compute(
                        "ReduceScatter", mybir.AluOpType.add,
                        ins=[part[:]], outs=[rso[:]],
                        replica_groups=[[0, 1, 2, 3], [4, 5, 6, 7]])
                    nc.sync.dma_start(out_d[qb], rso[:])

    nc.compile()
    return nc


def _get_nc():
    if "nc" not in _CACHE:
        _CACHE["nc"] = _build_nc()
    return _CACHE["nc"]


def _masks_np():
    k = np.arange(128)[:, None]
    q = np.arange(512)[None, :]
    return np.stack([(q >= k + 128 * o) for o in range(4)]).astype(BF16)


def kernel(x, Wq, Wk, Wv, Wp, bp):
    from concourse import bass_utils

    nc = _get_nc()
    x = np.asarray(x, dtype=np.float32)
    mk = _masks_np()

    # per-batch-group transposed x (shared by 4 cores each)
    xT_bg = []
    for bg in range(2):
        xl = x[2 * bg:2 * bg + 2].reshape(R, D)
        xT_bg.append(np.ascontiguousarray(xl.T).astype(BF16))
    # per-head-group weights
    wqk_hg, wv_hg, wp_hg = [], [], []
    for hg in range(4):
        hs = slice(192 * hg, 192 * (hg + 1))
        wqk_hg.append(np.concatenate([Wq[:, hs], Wk[:, hs]], axis=1).astype(BF16))
        wv_hg.append(np.asarray(Wv[:, hs]).astype(BF16))
        wpe = np.zeros((256, D), np.float32)
        wpe[:192] = np.asarray(Wp)[hs, :]
        wpe[192] = np.asarray(bp, dtype=np.float32) / 4.0
        wp_hg.append(wpe.astype(BF16))

    in_maps = []
    for c in range(N_CORES):
        bg, hg = c // 4, c % 4
        in_maps.append({
            "xT": xT_bg[bg],
            "wqk": wqk_hg[hg],
            "wv": wv_hg[hg],
            "wp": wp_hg[hg],
            "mk": mk,
        })

    res = bass_utils.run_bass_kernel_spmd(nc, in_maps,
                                          core_ids=list(range(N_CORES)))

    out = np.empty((B, S, D), np.float32)
    for g in range(2):
        for j in range(4):
            sh = res.results[4 * g + j]["out"]  # [4, 256, D]
            bl, off = divmod(j, 2)
            for qb in range(4):
                out[2 * g + bl, 512 * qb + 256 * off:
                    512 * qb + 256 * off + 256] = sh[qb]
    return out


# revision 10
# speedup vs baseline: 1.0915x; 1.0915x over previous
"""Multi-head causal attention (B=4,S=2048,D=768,H=12,HD=64) on 8 Trainium2 cores.

Sharding: 4-way head tensor-parallel (3 heads/core) x 2-way batch data-parallel
(2 batches/core).  Core c: batch group bg=c//4 (batches 2bg,2bg+1), head group
hg=c%4 (heads 3hg..3hg+2).

Per-core device program (SPMD; per-core differences come only from data):
  1. q/k projections emitted transposed (qT,kT: [64 head-dim partitions, rows]);
     v projection row-major with an appended ones column per head (softmax
     denominator rides along the AV matmul as psum row 64).
  2. Causal attention computed transposed: S_T[k,q] = kT.T @ qT, so P=exp(S_T)
     feeds AV directly as lhsT-free rhs with no P transpose.  Softmax skips the
     running max (scores are O(1) at this problem's scale; exp is exact vs the
     reference since softmax is shift-invariant).  The two batches of one head
     run concurrently on the PE via 64-row tile packing.  AV accumulates
     ctxU_T[65, q512] = sum_k vE.T @ P_T (row 64 = denominator l).
     Normalize: rec = 1/l (DVE), broadcast across 64 partitions via a float32r
     ones-outer-product matmul (exact: multiply by 1.0), one fused DVE multiply.
  3. Per 512-row q-block (x2 batches = 1024-row chunk): AllToAll (bf16) within
     the 4-core head group redistributes ctx so each core holds all 768 context
     features for its own 256 output rows; local projection with the full Wp;
     bias added via a K=1 ones-outer-product matmul.
Host side only slices/casts/transposes inputs and concatenates output shards.
"""

import sys

if "/opt/trn_rl_repo" not in sys.path:
    sys.path.insert(0, "/opt/trn_rl_repo")

import numpy as np
import ml_dtypes

BF16 = ml_dtypes.bfloat16

B, S, D = 4, 2048, 768
H, HD = 12, 64
N_CORES = 8
BL = 2          # batches per core
HL = 3          # heads per core
R = BL * S      # 4096 rows per core
KSUB = D // 128  # 6

_CACHE = {}


def _build_nc():
    import concourse.bass as bass  # noqa: F401
    import concourse.tile as tile
    from concourse import bacc, mybir

    f32 = mybir.dt.float32
    f32r = mybir.dt.float32r
    bf16 = mybir.dt.bfloat16
    EXP = mybir.ActivationFunctionType.Exp

    nc = bacc.Bacc("TRN2", target_bir_lowering=False, debug=False,
                   num_devices=N_CORES)

    xT_d = nc.dram_tensor("xT", [D, R], bf16, kind="ExternalInput").ap()
    wqk_d = nc.dram_tensor("wqk", [D, 2 * HL * HD], bf16, kind="ExternalInput").ap()
    wv_d = nc.dram_tensor("wv", [D, HL * HD], bf16, kind="ExternalInput").ap()
    wp_d = nc.dram_tensor("wp", [D, D], bf16, kind="ExternalInput").ap()
    bp_d = nc.dram_tensor("bp", [1, D], bf16, kind="ExternalInput").ap()
    mk_d = nc.dram_tensor("mk", [2, 128, 1024], bf16, kind="ExternalInput").ap()
    on_d = nc.dram_tensor("on64", [1, 64], f32r, kind="ExternalInput").ap()
    out_d = nc.dram_tensor("out", [4, 2, 128, D], f32, kind="ExternalOutput").ap()

    RG = [[0, 1, 2, 3, 4, 5, 6, 7]]

    with tile.TileContext(nc) as tc:
        with tc.tile_pool(name="persist", bufs=1) as per, \
             tc.tile_pool(name="dram", bufs=1, space="DRAM") as dram:
            # ---- persistent SBUF tensors -------------------------------
            wqk = per.tile([128, KSUB, 2 * HL * HD], bf16, tag="wqk")
            nc.sync.dma_start(wqk[:], wqk_d.rearrange("(o p) c -> p o c", p=128))
            wv = per.tile([128, KSUB, HL * HD], bf16, tag="wv")
            nc.sync.dma_start(wv[:], wv_d.rearrange("(o p) c -> p o c", p=128))
            xT = per.tile([128, KSUB, R], bf16, tag="xT")
            xTr = xT_d.rearrange("(o p) r -> p o r", p=128)
            for rc in range(8):  # 512-row chunks, all ksubs per chunk
                r0 = rc * 512
                nc.sync.dma_start(xT[:, :, r0:r0 + 512], xTr[:, :, r0:r0 + 512])
            masks = per.tile([128, 2, 1024], bf16, tag="mk")
            nc.sync.dma_start(masks[:], mk_d.rearrange("o p c -> p o c"))
            wp = per.tile([128, KSUB, D], bf16, tag="wp")
            nc.sync.dma_start(wp[:], wp_d.rearrange("(o p) c -> p o c", p=128))
            bp_sb = per.tile([1, D], bf16, tag="bp")
            nc.sync.dma_start(bp_sb[:], bp_d[:])
            ones64 = per.tile([1, 64], f32r, tag="ones64")
            nc.sync.dma_start(ones64[:], on_d[:])
            onesP = per.tile([1, 128], bf16, tag="onesP")
            nc.vector.memset(onesP[:], 1.0)

            # pair p = head p; partitions 0:64 = batch 0, 64:128 = batch 1
            qT = per.tile([128, HL, S], bf16, tag="qT")
            kT = per.tile([128, HL, S], bf16, tag="kT")
            # v row-tiles with ones col per head: [.., t, 65*h+64] == 1
            vE = per.tile([128, 2 * 16, HL * 65], bf16, tag="vE")
            nc.vector.memset(vE[:], 1.0)

            # ---- phase 1: projections ---------------------------------
            with tc.tile_pool(name="pj_ps", bufs=2, space="PSUM") as pj_ps:
                for b in range(BL):
                    for rc in range(4):  # 512-row chunks
                        r0 = b * S + rc * 512
                        for ct in range(3):  # qk col tiles of 128
                            ps = pj_ps.tile([128, 512], f32, tag="qkps")
                            for j in range(KSUB):
                                nc.tensor.matmul(
                                    ps[:],
                                    lhsT=wqk[:, j, ct * 128:(ct + 1) * 128],
                                    rhs=xT[:, j, r0:r0 + 512],
                                    start=(j == 0), stop=(j == KSUB - 1))
                            for half in range(2):
                                gid = 2 * ct + half
                                dest = qT if gid < 3 else kT
                                pair = gid % 3
                                nc.vector.tensor_copy(
                                    dest[b * 64:(b + 1) * 64, pair,
                                         rc * 512:(rc + 1) * 512],
                                    ps[half * 64:(half + 1) * 64, :])
                    for rt in range(16):  # v proj row tiles of 128
                        r0 = b * S + rt * 128
                        psv = pj_ps.tile([128, HL * HD], f32, tag="vps")
                        for j in range(KSUB):
                            nc.tensor.matmul(
                                psv[:],
                                lhsT=xT[:, j, r0:r0 + 128],
                                rhs=wv[:, j, :],
                                start=(j == 0), stop=(j == KSUB - 1))
                        for h in range(HL):
                            nc.vector.tensor_copy(
                                vE[:, b * 16 + rt, h * 65:h * 65 + 64],
                                psv[:, h * 64:(h + 1) * 64])

            # ---- phase 2: attention + A2A + out-proj -------------------
            with tc.tile_pool(name="st_ps", bufs=2, space="PSUM") as st_ps, \
                 tc.tile_pool(name="av_ps", bufs=2, space="PSUM") as av_ps, \
                 tc.tile_pool(name="bc_ps", bufs=1, space="PSUM") as bc_ps, \
                 tc.tile_pool(name="op_ps", bufs=1, space="PSUM") as op_ps, \
                 tc.tile_pool(name="pt", bufs=6) as ptp, \
                 tc.tile_pool(name="sm", bufs=4) as sm, \
                 tc.tile_pool(name="agp", bufs=2) as agp, \
                 tc.tile_pool(name="outp", bufs=2) as outp:
                for qb in range(4):
                    n_k = 4 * (qb + 1)
                    q0 = qb * 512
                    # 8 shards of [192 feats, 128 rows]: shard j = my features
                    # for rows [128j, 128j+128) of my 1024-row chunk
                    a2a_in = dram.tile([8 * 192, 128], bf16, tag=f"a2ai{qb}")
                    a2a_out = dram.tile([8 * 192, 128], bf16, tag=f"a2ao{qb}")
                    for pair in range(HL):
                        avs = [av_ps.tile([65, 512], f32, tag="av",
                                          name=f"av{u}")
                               for u in range(2)]
                        for kp in range(n_k // 2):  # k-tile pairs
                            for u in range(2):
                                stp = st_ps.tile([128, 2, 512], f32, tag="st")
                                for i in range(2):
                                    kt = 2 * kp + i
                                    nc.tensor.matmul(
                                        stp[:, i, :],
                                        lhsT=kT[u * 64:(u + 1) * 64, pair,
                                                kt * 128:(kt + 1) * 128],
                                        rhs=qT[u * 64:(u + 1) * 64, pair,
                                               q0:q0 + 512],
                                        start=True, stop=True)
                                pt = ptp.tile([128, 2, 512], bf16, tag="pt")
                                nc.scalar.activation(pt[:], stp[:], EXP,
                                                     scale=float(HD) ** -0.5)
                                if kp >= n_k // 2 - 2:
                                    o = kp - (n_k // 2 - 2)
                                    nc.vector.tensor_mul(pt[:], pt[:],
                                                         masks[:, o, :]
                                                         .rearrange("p (i c) -> p i c", i=2))
                                for i in range(2):
                                    kt = 2 * kp + i
                                    nc.tensor.matmul(
                                        avs[u][:],
                                        lhsT=vE[:, u * 16 + kt,
                                                pair * 65:(pair + 1) * 65],
                                        rhs=pt[:, i, :],
                                        start=(kp == 0 and i == 0),
                                        stop=(kp == n_k // 2 - 1 and i == 1))
                        for u in range(2):
                            rec = sm.tile([1, 512], f32r, tag="rec")
                            with nc.allow_low_precision(
                                    reason="float32r==fp32 bits; dtype tag only"):
                                nc.vector.reciprocal(rec[:], avs[u][64:65, :])
                            bc = bc_ps.tile([64, 512], f32, tag="bc")
                            nc.tensor.matmul(bc[:], lhsT=ones64[:], rhs=rec[:],
                                             start=True, stop=True)
                            bcs = sm.tile([64, 512], f32, tag="bcs")
                            nc.vector.tensor_copy(bcs[:], bc[:])
                            ctxn = sm.tile([64, 512], bf16, tag="ctxn")
                            nc.vector.tensor_mul(ctxn[:], avs[u][0:64, :], bcs[:])
                            for qq in range(4):
                                j = 4 * u + qq
                                nc.sync.dma_start(
                                    a2a_in[192 * j + 64 * pair:
                                           192 * j + 64 * (pair + 1), :],
                                    ctxn[:, qq * 128:(qq + 1) * 128])
                    nc.gpsimd.collective_compute(
                        "AllToAll", mybir.AluOpType.bypass,
                        ins=[a2a_in[:]], outs=[a2a_out[:]],
                        replica_groups=RG)
                    # a2a_out: [bp0 768 feats | bp1 768 feats] x 128 rows
                    ag = agp.tile([128, 2 * KSUB, 128], bf16, tag="ag")
                    nc.sync.dma_start(
                        ag[:], a2a_out.rearrange("(o p) r -> p o r", p=128))
                    for blk in range(2):  # batch-pair blocks
                        osb = outp.tile([128, D], f32, tag="osb")
                        for nh in range(2):
                            po = op_ps.tile([128, 384], f32, tag="op")
                            n0 = nh * 384
                            for j in range(KSUB):
                                nc.tensor.matmul(po[:],
                                                 lhsT=ag[:, blk * KSUB + j, :],
                                                 rhs=wp[:, j, n0:n0 + 384],
                                                 start=(j == 0), stop=False)
                            nc.tensor.matmul(po[:], lhsT=onesP[:],
                                             rhs=bp_sb[:, n0:n0 + 384],
                                             start=False, stop=True)
                            nc.vector.tensor_copy(osb[:, n0:n0 + 384], po[:])
                        nc.sync.dma_start(out_d[qb, blk], osb[:])

    nc.compile()
    return nc


def _get_nc():
    if "nc" not in _CACHE:
        _CACHE["nc"] = _build_nc()
    return _CACHE["nc"]


def _masks_np():
    # mask[o][k, 1024] covering diag k-tile pair offsets; o in {0,1} selects
    # k-tile pair (2o, 2o+1) relative to the last 4 diag tiles.
    k = np.arange(128)[:, None]
    q = np.arange(512)[None, :]
    tiles = [(q >= k + 128 * t) for t in range(4)]  # per diag k-tile offset t
    m = np.stack([np.concatenate([tiles[2 * o], tiles[2 * o + 1]], axis=1)
                  for o in range(2)])
    return m.astype(BF16)


def _prep_in_maps(x, Wq, Wk, Wv, Wp, bp):
    x = np.asarray(x, dtype=np.float32)
    mk = _masks_np()
    wp_full = np.asarray(Wp).astype(BF16)
    bp_row = np.asarray(bp, dtype=np.float32).reshape(1, D).astype(BF16)
    xT_bg = []
    for bg in range(2):
        xl = x[2 * bg:2 * bg + 2].reshape(R, D)
        xT_bg.append(np.ascontiguousarray(xl.T).astype(BF16))
    wqk_hg, wv_hg = [], []
    for hg in range(4):
        hs = slice(192 * hg, 192 * (hg + 1))
        wqk_hg.append(np.concatenate(
            [np.asarray(Wq)[:, hs], np.asarray(Wk)[:, hs]], axis=1).astype(BF16))
        wv_hg.append(np.asarray(Wv)[:, hs].astype(BF16))
    in_maps = []
    for c in range(N_CORES):
        bg, hg = c // 4, c % 4
        in_maps.append({
            "xT": xT_bg[bg],
            "wqk": wqk_hg[hg],
            "wv": wv_hg[hg],
            "wp": wp_full,
            "bp": bp_row,
            "on64": np.ones((1, 64), np.float32),
            "mk": mk,
        })
    return in_maps


def kernel(x, Wq, Wk, Wv, Wp, bp):
    from concourse import bass_utils

    nc = _get_nc()
    in_maps = _prep_in_maps(x, Wq, Wk, Wv, Wp, bp)
    res = bass_utils.run_bass_kernel_spmd(nc, in_maps,
                                          core_ids=list(range(N_CORES)))
    out = np.empty((B, S, D), np.float32)
    for c in range(N_CORES):
        sh = res.results[c]["out"]  # [4 chunks, 2 blocks, 128, D]
        for qb in range(4):
            for blk in range(2):
                batch = 2 * blk + c // 4
                s0 = 512 * qb + 128 * (c % 4)
                out[batch, s0:s0 + 128] = sh[qb, blk]
    return out


# revision 12
# speedup vs baseline: 1.1970x; 1.0967x over previous
"""Multi-head causal attention (B=4,S=2048,D=768,H=12,HD=64) on 8 Trainium2 cores.

Sharding: 4-way head tensor-parallel (3 heads/core) x 2-way batch data-parallel
(2 batches/core).  Core c: batch group bg=c//4 (batches 2bg,2bg+1), head group
hg=c%4 (heads 3hg..3hg+2).

Per-core device program (SPMD; per-core differences come only from data):
  1. q/k projections emitted transposed (qT,kT: [64 head-dim partitions, rows]);
     v projection row-major with an appended ones column per head (softmax
     denominator rides along the AV matmul as psum row 64).
  2. Causal attention computed transposed: S_T[k,q] = kT.T @ qT, so P=exp(S_T)
     feeds AV directly as lhsT-free rhs with no P transpose.  Softmax skips the
     running max (scores are O(1) at this problem's scale; exp is exact vs the
     reference since softmax is shift-invariant).  The two batches of one head
     run concurrently on the PE via 64-row tile packing.  AV accumulates
     ctxU_T[65, q512] = sum_k vE.T @ P_T (row 64 = denominator l).
     Normalize: rec = 1/l (DVE), broadcast across 64 partitions via a float32r
     ones-outer-product matmul (exact: multiply by 1.0), one fused DVE multiply.
  3. Per 512-row q-block (x2 batches = 1024-row chunk): AllToAll (bf16) within
     the 4-core head group redistributes ctx so each core holds all 768 context
     features for its own 256 output rows; local projection with the full Wp;
     bias added via a K=1 ones-outer-product matmul.
Host side only slices/casts/transposes inputs and concatenates output shards.
"""

import sys

if "/opt/trn_rl_repo" not in sys.path:
    sys.path.insert(0, "/opt/trn_rl_repo")

import numpy as np
import ml_dtypes

BF16 = ml_dtypes.bfloat16

B, S, D = 4, 2048, 768
H, HD = 12, 64
N_CORES = 8
BL = 2          # batches per core
HL = 3          # heads per core
R = BL * S      # 4096 rows per core
KSUB = D // 128  # 6

_CACHE = {}


def _build_nc():
    import concourse.bass as bass  # noqa: F401
    import concourse.tile as tile
    from concourse import bacc, mybir

    f32 = mybir.dt.float32
    f32r = mybir.dt.float32r
    bf16 = mybir.dt.bfloat16
    EXP = mybir.ActivationFunctionType.Exp

    nc = bacc.Bacc("TRN2", target_bir_lowering=False, debug=False,
                   num_devices=N_CORES)

    xT_d = nc.dram_tensor("xT", [D, R], bf16, kind="ExternalInput").ap()
    wqk_d = nc.dram_tensor("wqk", [D, 2 * HL * HD], bf16, kind="ExternalInput").ap()
    wv_d = nc.dram_tensor("wv", [D, HL * HD], bf16, kind="ExternalInput").ap()
    wp_d = nc.dram_tensor("wp", [D, D], bf16, kind="ExternalInput").ap()
    bp_d = nc.dram_tensor("bp", [1, D], bf16, kind="ExternalInput").ap()
    mk_d = nc.dram_tensor("mk", [2, 128, 1024], bf16, kind="ExternalInput").ap()
    out_d = nc.dram_tensor("out", [4, 2, 128, D], f32, kind="ExternalOutput").ap()

    RG = [[0, 1, 2, 3, 4, 5, 6, 7]]

    with tile.TileContext(nc) as tc:
        with tc.tile_pool(name="persist", bufs=1) as per, \
             tc.tile_pool(name="dram", bufs=1, space="DRAM") as dram:
            # ---- persistent SBUF tensors -------------------------------
            wqk = per.tile([128, KSUB, 2 * HL * HD], bf16, tag="wqk")
            nc.sync.dma_start(wqk[:], wqk_d.rearrange("(o p) c -> p o c", p=128))
            wv = per.tile([128, KSUB, HL * HD], bf16, tag="wv")
            nc.sync.dma_start(wv[:], wv_d.rearrange("(o p) c -> p o c", p=128))
            xT = per.tile([128, KSUB, R], bf16, tag="xT")
            xTr = xT_d.rearrange("(o p) r -> p o r", p=128)
            for rc in range(8):  # 512-row chunks, all ksubs per chunk
                r0 = rc * 512
                nc.sync.dma_start(xT[:, :, r0:r0 + 512], xTr[:, :, r0:r0 + 512])
            masks = per.tile([128, 2, 1024], bf16, tag="mk")
            nc.sync.dma_start(masks[:], mk_d.rearrange("o p c -> p o c"))
            wp = per.tile([128, KSUB, D], bf16, tag="wp")
            nc.sync.dma_start(wp[:], wp_d.rearrange("(o p) c -> p o c", p=128))
            bp_sb = per.tile([1, D], bf16, tag="bp")
            nc.sync.dma_start(bp_sb[:], bp_d[:])
            onesP = per.tile([1, 128], bf16, tag="onesP")
            nc.vector.memset(onesP[:], 1.0)

            # pair p = head p; partitions 0:64 = batch 0, 64:128 = batch 1
            qT = per.tile([128, HL, S], bf16, tag="qT")
            kT = per.tile([128, HL, S], bf16, tag="kT")
            # v row-tiles with ones col per head: [.., t, 65*h+64] == 1
            vE = per.tile([128, 2 * 16, HL * 65], bf16, tag="vE")
            nc.vector.memset(vE[:], 1.0)

            # ---- phases share PSUM so the scheduler can interleave ----
            with tc.tile_pool(name="mix_ps", bufs=2, space="PSUM") as mix_ps, \
                 tc.tile_pool(name="st_ps", bufs=2, space="PSUM") as st_ps, \
                 tc.tile_pool(name="av_ps", bufs=2, space="PSUM") as av_ps, \
                 tc.tile_pool(name="pt", bufs=6) as ptp, \
                 tc.tile_pool(name="sm", bufs=4) as sm, \
                 tc.tile_pool(name="agp", bufs=2) as agp, \
                 tc.tile_pool(name="outp", bufs=2) as outp:
                for b in range(BL):
                    for rc in range(4):  # 512-row chunks
                        r0 = b * S + rc * 512
                        for ct in range(3):  # qk col tiles of 128
                            ps = mix_ps.tile([128, 512], f32, tag="mix", name="ps")
                            for j in range(KSUB):
                                nc.tensor.matmul(
                                    ps[:],
                                    lhsT=wqk[:, j, ct * 128:(ct + 1) * 128],
                                    rhs=xT[:, j, r0:r0 + 512],
                                    start=(j == 0), stop=(j == KSUB - 1))
                            for half in range(2):
                                gid = 2 * ct + half
                                dest = qT if gid < 3 else kT
                                pair = gid % 3
                                nc.vector.tensor_copy(
                                    dest[b * 64:(b + 1) * 64, pair,
                                         rc * 512:(rc + 1) * 512],
                                    ps[half * 64:(half + 1) * 64, :])
                    for rt in range(16):  # v proj row tiles of 128
                        r0 = b * S + rt * 128
                        psv = mix_ps.tile([128, HL * HD], f32, tag="mix", name="psv")
                        for j in range(KSUB):
                            nc.tensor.matmul(
                                psv[:],
                                lhsT=xT[:, j, r0:r0 + 128],
                                rhs=wv[:, j, :],
                                start=(j == 0), stop=(j == KSUB - 1))
                        for h in range(HL):
                            nc.vector.tensor_copy(
                                vE[:, b * 16 + rt, h * 65:h * 65 + 64],
                                psv[:, h * 64:(h + 1) * 64])

                # ---- attention + A2A + out-proj ------------------------
                for qb in range(4):
                    n_k = 4 * (qb + 1)
                    q0 = qb * 512
                    # 8 shards of [192 feats, 128 rows]: shard j = my features
                    # for rows [128j, 128j+128) of my 1024-row chunk
                    a2a_in = dram.tile([8 * 192, 128], bf16, tag=f"a2ai{qb}")
                    a2a_out = dram.tile([8 * 192, 128], bf16, tag=f"a2ao{qb}")
                    for pair in range(HL):
                        avs = [av_ps.tile([65, 512], f32, tag="av",
                                          name=f"av{u}")
                               for u in range(2)]
                        for kp in range(n_k // 2):  # k-tile pairs
                            stps = [st_ps.tile([128, 2, 512], f32, tag="st",
                                               name=f"st{u}") for u in range(2)]
                            for i in range(2):
                                for u in range(2):
                                    kt = 2 * kp + i
                                    nc.tensor.matmul(
                                        stps[u][:, i, :],
                                        lhsT=kT[u * 64:(u + 1) * 64, pair,
                                                kt * 128:(kt + 1) * 128],
                                        rhs=qT[u * 64:(u + 1) * 64, pair,
                                               q0:q0 + 512],
                                        start=True, stop=True)
                            for u in range(2):
                                stp = stps[u]
                                pt = ptp.tile([128, 2, 512], bf16, tag="pt")
                                nc.scalar.activation(pt[:], stp[:], EXP,
                                                     scale=float(HD) ** -0.5)
                                if kp >= n_k // 2 - 2:
                                    o = kp - (n_k // 2 - 2)
                                    nc.vector.tensor_mul(pt[:], pt[:],
                                                         masks[:, o, :]
                                                         .rearrange("p (i c) -> p i c", i=2))
                                for i in range(2):
                                    kt = 2 * kp + i
                                    nc.tensor.matmul(
                                        avs[u][:],
                                        lhsT=vE[:, u * 16 + kt,
                                                pair * 65:(pair + 1) * 65],
                                        rhs=pt[:, i, :],
                                        start=(kp == 0 and i == 0),
                                        stop=(kp == n_k // 2 - 1 and i == 1))
                        for u in range(2):
                            lsb = sm.tile([1, 512], f32, tag="lsb")
                            nc.vector.tensor_copy(lsb[:], avs[u][64:65, :])
                            rec = sm.tile([1, 512], f32, tag="rec")
                            nc.vector.reciprocal_approx_fast(rec[:], lsb[:])
                            bcs = sm.tile([64, 512], f32, tag="bcs")
                            nc.gpsimd.partition_broadcast(bcs[:], rec[:])
                            ctxn = sm.tile([64, 512], bf16, tag="ctxn")
                            nc.vector.tensor_mul(ctxn[:], avs[u][0:64, :], bcs[:])
                            for qq in range(4):
                                j = 4 * u + qq
                                nc.sync.dma_start(
                                    a2a_in[192 * j + 64 * pair:
                                           192 * j + 64 * (pair + 1), :],
                                    ctxn[:, qq * 128:(qq + 1) * 128])
                    nc.gpsimd.collective_compute(
                        "AllToAll", mybir.AluOpType.bypass,
                        ins=[a2a_in[:]], outs=[a2a_out[:]],
                        replica_groups=RG)
                    # a2a_out: [bp0 768 feats | bp1 768 feats] x 128 rows
                    ag = agp.tile([128, 2 * KSUB, 128], bf16, tag="ag")
                    nc.sync.dma_start(
                        ag[:], a2a_out.rearrange("(o p) r -> p o r", p=128))
                    for blk in range(2):  # batch-pair blocks
                        osb = outp.tile([128, D], f32, tag="osb")
                        for nh in range(2):
                            po = mix_ps.tile([128, 384], f32, tag="mix", name="po")
                            n0 = nh * 384
                            for j in range(KSUB):
                                nc.tensor.matmul(po[:],
                                                 lhsT=ag[:, blk * KSUB + j, :],
                                                 rhs=wp[:, j, n0:n0 + 384],
                                                 start=(j == 0), stop=False)
                            nc.tensor.matmul(po[:], lhsT=onesP[:],
                                             rhs=bp_sb[:, n0:n0 + 384],
                                             start=False, stop=True)
                            nc.vector.tensor_copy(osb[:, n0:n0 + 384], po[:])
                        nc.sync.dma_start(out_d[qb, blk], osb[:])

    nc.compile()
    return nc


def _get_nc():
    if "nc" not in _CACHE:
        _CACHE["nc"] = _build_nc()
    return _CACHE["nc"]


def _masks_np():
    # mask[o][k, 1024] covering diag k-tile pair offsets; o in {0,1} selects
    # k-tile pair (2o, 2o+1) relative to the last 4 diag tiles.
    k = np.arange(128)[:, None]
    q = np.arange(512)[None, :]
    tiles = [(q >= k + 128 * t) for t in range(4)]  # per diag k-tile offset t
    m = np.stack([np.concatenate([tiles[2 * o], tiles[2 * o + 1]], axis=1)
                  for o in range(2)])
    return m.astype(BF16)


def _prep_in_maps(x, Wq, Wk, Wv, Wp, bp):
    x = np.asarray(x, dtype=np.float32)
    mk = _masks_np()
    wp_full = np.asarray(Wp).astype(BF16)
    bp_row = np.asarray(bp, dtype=np.float32).reshape(1, D).astype(BF16)
    xT_bg = []
    for bg in range(2):
        xl = x[2 * bg:2 * bg + 2].reshape(R, D)
        xT_bg.append(np.ascontiguousarray(xl.T).astype(BF16))
    wqk_hg, wv_hg = [], []
    for hg in range(4):
        hs = slice(192 * hg, 192 * (hg + 1))
        wqk_hg.append(np.concatenate(
            [np.asarray(Wq)[:, hs], np.asarray(Wk)[:, hs]], axis=1).astype(BF16))
        wv_hg.append(np.asarray(Wv)[:, hs].astype(BF16))
    in_maps = []
    for c in range(N_CORES):
        bg, hg = c // 4, c % 4
        in_maps.append({
            "xT": xT_bg[bg],
            "wqk": wqk_hg[hg],
            "wv": wv_hg[hg],
            "wp": wp_full,
            "bp": bp_row,
            "mk": mk,
        })
    return in_maps


def kernel(x, Wq, Wk, Wv, Wp, bp):
    from concourse import bass_utils

    nc = _get_nc()
    in_maps = _prep_in_maps(x, Wq, Wk, Wv, Wp, bp)
    res = bass_utils.run_bass_kernel_spmd(nc, in_maps,
                                          core_ids=list(range(N_CORES)))
    out = np.empty((B, S, D), np.float32)
    for c in range(N_CORES):
        sh = res.results[c]["out"]  # [4 chunks, 2 blocks, 128, D]
        for qb in range(4):
            for blk in range(2):
                batch = 2 * blk + c // 4
                s0 = 512 * qb + 128 * (c % 4)
                out[batch, s0:s0 + 128] = sh[qb, blk]
    return out


# revision 13
# speedup vs baseline: 1.2092x; 1.0102x over previous
"""Multi-head causal attention (B=4,S=2048,D=768,H=12,HD=64) on 8 Trainium2 cores.

Sharding: 4-way head tensor-parallel (3 heads/core) x 2-way batch data-parallel
(2 batches/core).  Core c: batch group bg=c//4 (batches 2bg,2bg+1), head group
hg=c%4 (heads 3hg..3hg+2).

Per-core device program (SPMD; per-core differences come only from data):
  1. q/k projections emitted transposed (qT,kT: [64 head-dim partitions, rows]);
     v projection row-major with an appended ones column per head (softmax
     denominator rides along the AV matmul as psum row 64).
  2. Causal attention computed transposed: S_T[k,q] = kT.T @ qT, so P=exp(S_T)
     feeds AV directly with no P transpose.  Softmax skips the running max
     (scores are O(1) at this problem's scale; exp is mathematically identical
     to the reference since softmax is shift-invariant).  The two batches of a
     head run concurrently on the PE via 64-row tile packing.  AV accumulates
     ctxU_T[65, q512] = sum_k vE.T @ P_T (row 64 = denominator l).  Normalize:
     1/l via fast-approx DVE reciprocal, broadcast across partitions on GpSimd,
     one fused DVE multiply.
  3. Per 512-row q-block (x2 batches = 1024-row chunk): 8-core AllToAll (bf16,
     128-row shards) redistributes ctx so each core holds all 768 context
     features for its own 2x128 output rows; local projection with full Wp;
     bias via a K=1 ones-outer-product matmul.

Emission is software-pipelined: qk/v projection chunks are interleaved between
attention pairs as PE filler (keeps the PE dense so HAM stays at full clock),
and each chunk's out-projection is emitted one q-block later so the PE never
head-of-line blocks on the AllToAll round trip.

Host side only slices/casts/transposes inputs and concatenates output shards.
"""

import sys

if "/opt/trn_rl_repo" not in sys.path:
    sys.path.insert(0, "/opt/trn_rl_repo")

import numpy as np
import ml_dtypes

BF16 = ml_dtypes.bfloat16

B, S, D = 4, 2048, 768
H, HD = 12, 64
N_CORES = 8
BL = 2          # batches per core
HL = 3          # heads per core
R = BL * S      # 4096 rows per core
KSUB = D // 128  # 6

_CACHE = {}


def _build_nc():
    import concourse.bass as bass  # noqa: F401
    import concourse.tile as tile
    from concourse import bacc, mybir

    f32 = mybir.dt.float32
    bf16 = mybir.dt.bfloat16
    EXP = mybir.ActivationFunctionType.Exp

    nc = bacc.Bacc("TRN2", target_bir_lowering=False, debug=False,
                   num_devices=N_CORES)

    xT_d = nc.dram_tensor("xT", [D, R], bf16, kind="ExternalInput").ap()
    wqk_d = nc.dram_tensor("wqk", [D, 2 * HL * HD], bf16, kind="ExternalInput").ap()
    wv_d = nc.dram_tensor("wv", [D, HL * HD], bf16, kind="ExternalInput").ap()
    wp_d = nc.dram_tensor("wp", [D, D], bf16, kind="ExternalInput").ap()
    bp_d = nc.dram_tensor("bp", [1, D], bf16, kind="ExternalInput").ap()
    mk_d = nc.dram_tensor("mk", [2, 128, 1024], bf16, kind="ExternalInput").ap()
    out_d = nc.dram_tensor("out", [4, 2, 128, D], f32, kind="ExternalOutput").ap()

    RG = [[0, 1, 2, 3, 4, 5, 6, 7]]

    with tile.TileContext(nc) as tc:
        with tc.tile_pool(name="persist", bufs=1) as per, \
             tc.tile_pool(name="dram", bufs=1, space="DRAM") as dram, \
             tc.tile_pool(name="mix_ps", bufs=2, space="PSUM") as mix_ps, \
             tc.tile_pool(name="st_ps", bufs=2, space="PSUM") as st_ps, \
             tc.tile_pool(name="av_ps", bufs=2, space="PSUM") as av_ps, \
             tc.tile_pool(name="pt", bufs=6) as ptp, \
             tc.tile_pool(name="sm", bufs=4) as sm, \
             tc.tile_pool(name="agp", bufs=2) as agp, \
             tc.tile_pool(name="outp", bufs=2) as outp:
            # ---- persistent SBUF tensors -------------------------------
            wqk = per.tile([128, KSUB, 2 * HL * HD], bf16, tag="wqk")
            nc.sync.dma_start(wqk[:], wqk_d.rearrange("(o p) c -> p o c", p=128))
            wv = per.tile([128, KSUB, HL * HD], bf16, tag="wv")
            nc.sync.dma_start(wv[:], wv_d.rearrange("(o p) c -> p o c", p=128))
            xT = per.tile([128, KSUB, R], bf16, tag="xT")
            xTr = xT_d.rearrange("(o p) r -> p o r", p=128)
            for rc in range(8):  # per (row-chunk, ksub) pieces
                r0 = rc * 512
                for j in range(KSUB):
                    nc.sync.dma_start(xT[:, j, r0:r0 + 512],
                                      xTr[:, j, r0:r0 + 512])
            masks = per.tile([128, 2, 1024], bf16, tag="mk")
            nc.sync.dma_start(masks[:], mk_d.rearrange("o p c -> p o c"))
            wp = per.tile([128, KSUB, D], bf16, tag="wp")
            nc.sync.dma_start(wp[:], wp_d.rearrange("(o p) c -> p o c", p=128))
            bp_sb = per.tile([1, D], bf16, tag="bp")
            nc.sync.dma_start(bp_sb[:], bp_d[:])
            onesP = per.tile([1, 128], bf16, tag="onesP")
            nc.vector.memset(onesP[:], 1.0)

            # pair p = head p; partitions 0:64 = batch 0, 64:128 = batch 1
            qT = per.tile([128, HL, S], bf16, tag="qT")
            kT = per.tile([128, HL, S], bf16, tag="kT")
            vE = per.tile([128, 2 * 16, HL * 65], bf16, tag="vE")
            nc.vector.memset(vE[:], 1.0)

            a2a_in = [dram.tile([8 * 192, 128], bf16, name=f"a2ai{qb}",
                                tag=f"a2ai{qb}") for qb in range(4)]
            a2a_out = [dram.tile([8 * 192, 128], bf16, name=f"a2ao{qb}",
                                 tag=f"a2ao{qb}") for qb in range(4)]

            # ---- emission helpers --------------------------------------
            def emit_qk_chunk(b, rc):
                r0 = b * S + rc * 512
                for ct in range(3):
                    ps = mix_ps.tile([128, 512], f32, tag="mix", name="ps")
                    for j in range(KSUB):
                        nc.tensor.matmul(
                            ps[:],
                            lhsT=wqk[:, j, ct * 128:(ct + 1) * 128],
                            rhs=xT[:, j, r0:r0 + 512],
                            start=(j == 0), stop=(j == KSUB - 1))
                    for half in range(2):
                        gid = 2 * ct + half
                        dest = qT if gid < 3 else kT
                        pair = gid % 3
                        nc.vector.tensor_copy(
                            dest[b * 64:(b + 1) * 64, pair,
                                 rc * 512:(rc + 1) * 512],
                            ps[half * 64:(half + 1) * 64, :])

            def emit_v_tile(b, rt):
                r0 = b * S + rt * 128
                psv = mix_ps.tile([128, HL * HD], f32, tag="mix", name="psv")
                for j in range(KSUB):
                    nc.tensor.matmul(
                        psv[:], lhsT=xT[:, j, r0:r0 + 128], rhs=wv[:, j, :],
                        start=(j == 0), stop=(j == KSUB - 1))
                for h in range(HL):
                    nc.vector.tensor_copy(
                        vE[:, b * 16 + rt, h * 65:h * 65 + 64],
                        psv[:, h * 64:(h + 1) * 64])

            def emit_attn_pair(qb, pair):
                n_k = 4 * (qb + 1)
                q0 = qb * 512
                avs = [av_ps.tile([65, 512], f32, tag="av", name=f"av{u}")
                       for u in range(2)]
                for kp in range(n_k // 2):
                    stps = [st_ps.tile([128, 2, 512], f32, tag="st",
                                       name=f"st{u}") for u in range(2)]
                    for i in range(2):
                        for u in range(2):
                            kt = 2 * kp + i
                            nc.tensor.matmul(
                                stps[u][:, i, :],
                                lhsT=kT[u * 64:(u + 1) * 64, pair,
                                        kt * 128:(kt + 1) * 128],
                                rhs=qT[u * 64:(u + 1) * 64, pair, q0:q0 + 512],
                                start=True, stop=True)
                    for u in range(2):
                        pt = ptp.tile([128, 2, 512], bf16, tag="pt")
                        nc.scalar.activation(pt[:], stps[u][:], EXP,
                                             scale=float(HD) ** -0.5)
                        if kp >= n_k // 2 - 2:
                            o = kp - (n_k // 2 - 2)
                            nc.vector.tensor_mul(
                                pt[:], pt[:],
                                masks[:, o, :].rearrange("p (i c) -> p i c", i=2))
                        for i in range(2):
                            kt = 2 * kp + i
                            nc.tensor.matmul(
                                avs[u][:],
                                lhsT=vE[:, u * 16 + kt,
                                        pair * 65:(pair + 1) * 65],
                                rhs=pt[:, i, :],
                                start=(kp == 0 and i == 0),
                                stop=(kp == n_k // 2 - 1 and i == 1))
                for u in range(2):
                    lsb = sm.tile([1, 512], f32, tag="lsb")
                    nc.vector.tensor_copy(lsb[:], avs[u][64:65, :])
                    rec = sm.tile([1, 512], f32, tag="rec")
                    nc.vector.reciprocal_approx_fast(rec[:], lsb[:])
                    bcs = sm.tile([64, 512], f32, tag="bcs")
                    nc.gpsimd.partition_broadcast(bcs[:], rec[:])
                    ctxn = sm.tile([64, 512], bf16, tag="ctxn")
                    nc.vector.tensor_mul(ctxn[:], avs[u][0:64, :], bcs[:])
                    for qq in range(4):
                        j = 4 * u + qq
                        nc.sync.dma_start(
                            a2a_in[qb][192 * j + 64 * pair:
                                       192 * j + 64 * (pair + 1), :],
                            ctxn[:, qq * 128:(qq + 1) * 128])

            def emit_a2a(qb):
                nc.gpsimd.collective_compute(
                    "AllToAll", mybir.AluOpType.bypass,
                    ins=[a2a_in[qb][:]], outs=[a2a_out[qb][:]],
                    replica_groups=RG)

            def emit_outproj(qb):
                ag = agp.tile([128, 2 * KSUB, 128], bf16, tag="ag")
                nc.sync.dma_start(
                    ag[:], a2a_out[qb].rearrange("(o p) r -> p o r", p=128))
                for blk in range(2):
                    osb = outp.tile([128, D], f32, tag="osb")
                    for nh in range(2):
                        po = mix_ps.tile([128, 384], f32, tag="mix", name="po")
                        n0 = nh * 384
                        for j in range(KSUB):
                            nc.tensor.matmul(po[:],
                                             lhsT=ag[:, blk * KSUB + j, :],
                                             rhs=wp[:, j, n0:n0 + 384],
                                             start=(j == 0), stop=False)
                        nc.tensor.matmul(po[:], lhsT=onesP[:],
                                         rhs=bp_sb[:, n0:n0 + 384],
                                         start=False, stop=True)
                        nc.vector.tensor_copy(osb[:, n0:n0 + 384], po[:])
                    nc.sync.dma_start(out_d[qb, blk], osb[:])

            # ---- software-pipelined emission ---------------------------
            # prologue: everything attention qb0 needs
            emit_qk_chunk(0, 0)
            emit_qk_chunk(1, 0)
            for rt in range(4):
                emit_v_tile(0, rt)
                emit_v_tile(1, rt)
            # fillers[qb][pair]: PE work interleaved into qb's pair loop; all
            # of qb+1's projection inputs are emitted inside qb.
            fillers = {
                0: [lambda: emit_qk_chunk(0, 1), lambda: emit_qk_chunk(1, 1),
                    lambda: [emit_v_tile(b, rt) for b in range(2)
                             for rt in range(4, 8)]],
                1: [lambda: emit_qk_chunk(0, 2), lambda: emit_qk_chunk(1, 2),
                    lambda: [emit_v_tile(b, rt) for b in range(2)
                             for rt in range(8, 12)]],
                2: [lambda: emit_qk_chunk(0, 3), lambda: emit_qk_chunk(1, 3),
                    lambda: [emit_v_tile(b, rt) for b in range(2)
                             for rt in range(12, 16)]],
                3: [lambda: None, lambda: None, lambda: None],
            }
            for qb in range(4):
                for pair in range(HL):
                    emit_attn_pair(qb, pair)
                    fillers[qb][pair]()
                    if pair == 0 and qb >= 1:
                        emit_outproj(qb - 1)  # previous chunk, A2A done by now
                emit_a2a(qb)
            emit_outproj(3)

    nc.compile()
    return nc


def _get_nc():
    if "nc" not in _CACHE:
        _CACHE["nc"] = _build_nc()
    return _CACHE["nc"]


def _masks_np():
    k = np.arange(128)[:, None]
    q = np.arange(512)[None, :]
    tiles = [(q >= k + 128 * t) for t in range(4)]
    m = np.stack([np.concatenate([tiles[2 * o], tiles[2 * o + 1]], axis=1)
                  for o in range(2)])
    return m.astype(BF16)


def _prep_in_maps(x, Wq, Wk, Wv, Wp, bp):
    x = np.asarray(x, dtype=np.float32)
    mk = _masks_np()
    wp_full = np.asarray(Wp).astype(BF16)
    bp_row = np.asarray(bp, dtype=np.float32).reshape(1, D).astype(BF16)
    xT_bg = []
    for bg in range(2):
        xl = x[2 * bg:2 * bg + 2].reshape(R, D)
        xT_bg.append(np.ascontiguousarray(xl.T).astype(BF16))
    wqk_hg, wv_hg = [], []
    for hg in range(4):
        hs = slice(192 * hg, 192 * (hg + 1))
        wqk_hg.append(np.concatenate(
            [np.asarray(Wq)[:, hs], np.asarray(Wk)[:, hs]], axis=1).astype(BF16))
        wv_hg.append(np.asarray(Wv)[:, hs].astype(BF16))
    in_maps = []
    for c in range(N_CORES):
        bg, hg = c // 4, c % 4
        in_maps.append({
            "xT": xT_bg[bg],
            "wqk": wqk_hg[hg],
            "wv": wv_hg[hg],
            "wp": wp_full,
            "bp": bp_row,
            "mk": mk,
        })
    return in_maps


def kernel(x, Wq, Wk, Wv, Wp, bp):
    from concourse import bass_utils

    nc = _get_nc()
    in_maps = _prep_in_maps(x, Wq, Wk, Wv, Wp, bp)
    res = bass_utils.run_bass_kernel_spmd(nc, in_maps,
                                          core_ids=list(range(N_CORES)))
    out = np.empty((B, S, D), np.float32)
    for c in range(N_CORES):
        sh = res.results[c]["out"]  # [4 chunks, 2 blocks, 128, D]
        for qb in range(4):
            for blk in range(2):
                batch = 2 * blk + c // 4
                s0 = 512 * qb + 128 * (c % 4)
                out[batch, s0:s0 + 128] = sh[qb, blk]
    return out


# revision 14
# speedup vs baseline: 1.2752x; 1.0546x over previous
"""Multi-head causal attention (B=4,S=2048,D=768,H=12,HD=64) on 8 Trainium2 cores.

Sharding: 4-way head tensor-parallel (3 heads/core) x 2-way batch data-parallel
(2 batches/core).  Core c: batch group bg=c//4 (batches 2bg,2bg+1), head group
hg=c%4 (heads 3hg..3hg+2).

Per-core device program (SPMD; per-core differences come only from data):
  1. q/k projections emitted transposed (qT,kT: [64 head-dim partitions, rows]);
     v projection row-major with an appended ones column per head (softmax
     denominator rides along the AV matmul as psum row 64).
  2. Causal attention computed transposed: S_T[k,q] = kT.T @ qT, so P=exp(S_T)
     feeds AV directly with no P transpose.  Softmax skips the running max
     (scores are O(1) at this problem's scale; exp is mathematically identical
     to the reference since softmax is shift-invariant).  The two batches of a
     head run concurrently on the PE via 64-row tile packing.  AV accumulates
     ctxU_T[65, q512] = sum_k vE.T @ P_T (row 64 = denominator l).  Normalize:
     1/l via fast-approx DVE reciprocal, broadcast across partitions on GpSimd,
     one fused DVE multiply.
  3. Per 512-row q-block (x2 batches = 1024-row chunk): 8-core AllToAll (bf16,
     128-row shards) redistributes ctx so each core holds all 768 context
     features for its own 2x128 output rows; local projection with full Wp;
     bias via a K=1 ones-outer-product matmul.

Emission is software-pipelined: qk/v projection chunks are interleaved between
attention pairs as PE filler (keeps the PE dense so HAM stays at full clock),
and each chunk's out-projection is emitted one q-block later so the PE never
head-of-line blocks on the AllToAll round trip.

Host side only slices/casts/transposes inputs and concatenates output shards.
"""

import sys

if "/opt/trn_rl_repo" not in sys.path:
    sys.path.insert(0, "/opt/trn_rl_repo")

import numpy as np
import ml_dtypes

BF16 = ml_dtypes.bfloat16

B, S, D = 4, 2048, 768
H, HD = 12, 64
N_CORES = 8
BL = 2          # batches per core
HL = 3          # heads per core
R = BL * S      # 4096 rows per core
KSUB = D // 128  # 6

_CACHE = {}


def _build_nc():
    import concourse.bass as bass  # noqa: F401
    import concourse.tile as tile
    from concourse import bacc, mybir

    f32 = mybir.dt.float32
    bf16 = mybir.dt.bfloat16
    EXP = mybir.ActivationFunctionType.Exp

    nc = bacc.Bacc("TRN2", target_bir_lowering=False, debug=False,
                   num_devices=N_CORES)

    xT_d = nc.dram_tensor("xT", [D, R], bf16, kind="ExternalInput").ap()
    wqk_d = nc.dram_tensor("wqk", [D, 2 * HL * HD], bf16, kind="ExternalInput").ap()
    wv_d = nc.dram_tensor("wv", [D, HL * HD], bf16, kind="ExternalInput").ap()
    wp_d = nc.dram_tensor("wp", [D, D], bf16, kind="ExternalInput").ap()
    bp_d = nc.dram_tensor("bp", [1, D], bf16, kind="ExternalInput").ap()
    mk_d = nc.dram_tensor("mk", [2, 128, 1024], bf16, kind="ExternalInput").ap()
    out_d = nc.dram_tensor("out", [4, 2, 128, D], f32, kind="ExternalOutput").ap()

    RG = [[0, 1, 2, 3, 4, 5, 6, 7]]

    with tile.TileContext(nc) as tc:
        with tc.tile_pool(name="persist", bufs=1) as per, \
             tc.tile_pool(name="dram", bufs=1, space="DRAM") as dram, \
             tc.tile_pool(name="mix_ps", bufs=2, space="PSUM") as mix_ps, \
             tc.tile_pool(name="st_ps", bufs=2, space="PSUM") as st_ps, \
             tc.tile_pool(name="av_ps", bufs=2, space="PSUM") as av_ps, \
             tc.tile_pool(name="pt", bufs=6) as ptp, \
             tc.tile_pool(name="sm", bufs=4) as sm, \
             tc.tile_pool(name="agp", bufs=2) as agp, \
             tc.tile_pool(name="outp", bufs=2) as outp:
            # ---- persistent SBUF tensors -------------------------------
            wqk = per.tile([128, KSUB, 2 * HL * HD], bf16, tag="wqk")
            wv = per.tile([128, KSUB, HL * HD], bf16, tag="wv")
            for j in range(KSUB):
                nc.sync.dma_start(
                    wqk[:, j], wqk_d.rearrange("(o p) c -> p o c", p=128)[:, j])
                nc.sync.dma_start(
                    wv[:, j], wv_d.rearrange("(o p) c -> p o c", p=128)[:, j])
            xT = per.tile([128, KSUB, R], bf16, tag="xT")
            xTr = xT_d.rearrange("(o p) r -> p o r", p=128)
            for rc in range(8):  # per (row-chunk, ksub) pieces
                r0 = rc * 512
                for j in range(KSUB):
                    nc.sync.dma_start(xT[:, j, r0:r0 + 512],
                                      xTr[:, j, r0:r0 + 512])
            masks = per.tile([128, 2, 1024], bf16, tag="mk")
            nc.sync.dma_start(masks[:], mk_d.rearrange("o p c -> p o c"))
            wp = per.tile([128, KSUB, D], bf16, tag="wp")
            nc.sync.dma_start(wp[:], wp_d.rearrange("(o p) c -> p o c", p=128))
            bp_sb = per.tile([1, D], bf16, tag="bp")
            nc.sync.dma_start(bp_sb[:], bp_d[:])
            onesP = per.tile([1, 128], bf16, tag="onesP")
            nc.vector.memset(onesP[:], 1.0)

            # pair p = head p; partitions 0:64 = batch 0, 64:128 = batch 1
            qT = per.tile([128, HL, S], bf16, tag="qT")
            kT = per.tile([128, HL, S], bf16, tag="kT")
            vE = per.tile([128, 2 * 16, HL * 65], bf16, tag="vE")
            nc.vector.memset(vE[:], 1.0)

            warm_in = dram.tile([8 * 192, 8], bf16, tag="warm_in")
            warm_out = dram.tile([8 * 192, 8], bf16, tag="warm_out")
            a2a_in = [dram.tile([8 * 192, 128], bf16, name=f"a2ai{qb}",
                                tag=f"a2ai{qb}") for qb in range(4)]
            a2a_out = [dram.tile([8 * 192, 128], bf16, name=f"a2ao{qb}",
                                 tag=f"a2ao{qb}") for qb in range(4)]

            # ---- emission helpers --------------------------------------
            def emit_qk_chunk(b, rc):
                r0 = b * S + rc * 512
                for ct in range(3):
                    ps = mix_ps.tile([128, 512], f32, tag="mix", name="ps")
                    for j in range(KSUB):
                        nc.tensor.matmul(
                            ps[:],
                            lhsT=wqk[:, j, ct * 128:(ct + 1) * 128],
                            rhs=xT[:, j, r0:r0 + 512],
                            start=(j == 0), stop=(j == KSUB - 1))
                    for half in range(2):
                        gid = 2 * ct + half
                        dest = qT if gid < 3 else kT
                        pair = gid % 3
                        nc.vector.tensor_copy(
                            dest[b * 64:(b + 1) * 64, pair,
                                 rc * 512:(rc + 1) * 512],
                            ps[half * 64:(half + 1) * 64, :])

            def emit_v_tile(b, rt):
                r0 = b * S + rt * 128
                psv = mix_ps.tile([128, HL * HD], f32, tag="mix", name="psv")
                for j in range(KSUB):
                    nc.tensor.matmul(
                        psv[:], lhsT=xT[:, j, r0:r0 + 128], rhs=wv[:, j, :],
                        start=(j == 0), stop=(j == KSUB - 1))
                for h in range(HL):
                    nc.vector.tensor_copy(
                        vE[:, b * 16 + rt, h * 65:h * 65 + 64],
                        psv[:, h * 64:(h + 1) * 64])

            def emit_attn_pair(qb, pair):
                n_k = 4 * (qb + 1)
                q0 = qb * 512
                avs = [av_ps.tile([65, 512], f32, tag="av", name=f"av{u}")
                       for u in range(2)]
                for kp in range(n_k // 2):
                    stps = [st_ps.tile([128, 2, 512], f32, tag="st",
                                       name=f"st{u}") for u in range(2)]
                    for i in range(2):
                        for u in range(2):
                            kt = 2 * kp + i
                            nc.tensor.matmul(
                                stps[u][:, i, :],
                                lhsT=kT[u * 64:(u + 1) * 64, pair,
                                        kt * 128:(kt + 1) * 128],
                                rhs=qT[u * 64:(u + 1) * 64, pair, q0:q0 + 512],
                                start=True, stop=True)
                    for u in range(2):
                        pt = ptp.tile([128, 2, 512], bf16, tag="pt")
                        nc.scalar.activation(pt[:], stps[u][:], EXP,
                                             scale=float(HD) ** -0.5)
                        if kp >= n_k // 2 - 2:
                            o = kp - (n_k // 2 - 2)
                            nc.vector.tensor_mul(
                                pt[:], pt[:],
                                masks[:, o, :].rearrange("p (i c) -> p i c", i=2))
                        for i in range(2):
                            kt = 2 * kp + i
                            nc.tensor.matmul(
                                avs[u][:],
                                lhsT=vE[:, u * 16 + kt,
                                        pair * 65:(pair + 1) * 65],
                                rhs=pt[:, i, :],
                                start=(kp == 0 and i == 0),
                                stop=(kp == n_k // 2 - 1 and i == 1))
                for u in range(2):
                    lsb = sm.tile([1, 512], f32, tag="lsb")
                    nc.vector.tensor_copy(lsb[:], avs[u][64:65, :])
                    rec = sm.tile([1, 512], f32, tag="rec")
                    nc.vector.reciprocal_approx_fast(rec[:], lsb[:])
                    bcs = sm.tile([64, 512], f32, tag="bcs")
                    nc.gpsimd.partition_broadcast(bcs[:], rec[:])
                    ctxn = sm.tile([64, 512], bf16, tag="ctxn")
                    nc.vector.tensor_mul(ctxn[:], avs[u][0:64, :], bcs[:])
                    for qq in range(4):
                        j = 4 * u + qq
                        nc.sync.dma_start(
                            a2a_in[qb][192 * j + 64 * pair:
                                       192 * j + 64 * (pair + 1), :],
                            ctxn[:, qq * 128:(qq + 1) * 128])

            def emit_a2a(qb):
                nc.gpsimd.collective_compute(
                    "AllToAll", mybir.AluOpType.bypass,
                    ins=[a2a_in[qb][:]], outs=[a2a_out[qb][:]],
                    replica_groups=RG)

            def emit_outproj(qb):
                ag = agp.tile([128, 2 * KSUB, 128], bf16, tag="ag")
                nc.sync.dma_start(
                    ag[:], a2a_out[qb].rearrange("(o p) r -> p o r", p=128))
                for blk in range(2):
                    osb = outp.tile([128, D], f32, tag="osb")
                    for nh in range(2):
                        po = mix_ps.tile([128, 384], f32, tag="mix", name="po")
                        n0 = nh * 384
                        for j in range(KSUB):
                            nc.tensor.matmul(po[:],
                                             lhsT=ag[:, blk * KSUB + j, :],
                                             rhs=wp[:, j, n0:n0 + 384],
                                             start=(j == 0), stop=False)
                        nc.tensor.matmul(po[:], lhsT=onesP[:],
                                         rhs=bp_sb[:, n0:n0 + 384],
                                         start=False, stop=True)
                        nc.vector.tensor_copy(osb[:, n0:n0 + 384], po[:])
                    nc.sync.dma_start(out_d[qb, blk], osb[:])

            # ---- software-pipelined emission ---------------------------
            # warmup collective: absorb ncfw first-call overhead during proj
            nc.sync.dma_start(warm_in[0:128, :], masks[:, 0, 0:8])
            nc.gpsimd.collective_compute(
                "AllToAll", mybir.AluOpType.bypass,
                ins=[warm_in[:]], outs=[warm_out[:]], replica_groups=RG)
            # prologue: everything attention qb0 needs
            emit_qk_chunk(0, 0)
            emit_qk_chunk(1, 0)
            for rt in range(4):
                emit_v_tile(0, rt)
                emit_v_tile(1, rt)
            # fillers[qb][pair]: PE work interleaved into qb's pair loop; all
            # of qb+1's projection inputs are emitted inside qb.
            fillers = {
                0: [lambda: emit_qk_chunk(0, 1), lambda: emit_qk_chunk(1, 1),
                    lambda: [emit_v_tile(b, rt) for b in range(2)
                             for rt in range(4, 8)]],
                1: [lambda: emit_qk_chunk(0, 2), lambda: emit_qk_chunk(1, 2),
                    lambda: [emit_v_tile(b, rt) for b in range(2)
                             for rt in range(8, 12)]],
                2: [lambda: emit_qk_chunk(0, 3), lambda: emit_qk_chunk(1, 3),
                    lambda: [emit_v_tile(b, rt) for b in range(2)
                             for rt in range(12, 16)]],
                3: [lambda: None, lambda: None, lambda: None],
            }
            for qb in range(4):
                for pair in range(HL):
                    emit_attn_pair(qb, pair)
                    fillers[qb][pair]()
                    if pair == 1 and qb >= 1:
                        emit_outproj(qb - 1)  # previous chunk, A2A done by now
                emit_a2a(qb)
            emit_outproj(3)

    nc.compile()
    return nc


def _get_nc():
    if "nc" not in _CACHE:
        _CACHE["nc"] = _build_nc()
    return _CACHE["nc"]


def _masks_np():
    k = np.arange(128)[:, None]
    q = np.arange(512)[None, :]
    tiles = [(q >= k + 128 * t) for t in range(4)]
    m = np.stack([np.concatenate([tiles[2 * o], tiles[2 * o + 1]], axis=1)
                  for o in range(2)])
    return m.astype(BF16)


def _prep_in_maps(x, Wq, Wk, Wv, Wp, bp):
    x = np.asarray(x, dtype=np.float32)
    mk = _masks_np()
    wp_full = np.asarray(Wp).astype(BF16)
    bp_row = np.asarray(bp, dtype=np.float32).reshape(1, D).astype(BF16)
    xT_bg = []
    for bg in range(2):
        xl = x[2 * bg:2 * bg + 2].reshape(R, D)
        xT_bg.append(np.ascontiguousarray(xl.T).astype(BF16))
    wqk_hg, wv_hg = [], []
    for hg in range(4):
        hs = slice(192 * hg, 192 * (hg + 1))
        wqk_hg.append(np.concatenate(
            [np.asarray(Wq)[:, hs], np.asarray(Wk)[:, hs]], axis=1).astype(BF16))
        wv_hg.append(np.asarray(Wv)[:, hs].astype(BF16))
    in_maps = []
    for c in range(N_CORES):
        bg, hg = c // 4, c % 4
        in_maps.append({
            "xT": xT_bg[bg],
            "wqk": wqk_hg[hg],
            "wv": wv_hg[hg],
            "wp": wp_full,
            "bp": bp_row,
            "mk": mk,
        })
    return in_maps


def kernel(x, Wq, Wk, Wv, Wp, bp):
    from concourse import bass_utils

    nc = _get_nc()
    in_maps = _prep_in_maps(x, Wq, Wk, Wv, Wp, bp)
    res = bass_utils.run_bass_kernel_spmd(nc, in_maps,
                                          core_ids=list(range(N_CORES)))
    out = np.empty((B, S, D), np.float32)
    for c in range(N_CORES):
        sh = res.results[c]["out"]  # [4 chunks, 2 blocks, 128, D]
        for qb in range(4):
            for blk in range(2):
                batch = 2 * blk + c // 4
                s0 = 512 * qb + 128 * (c % 4)
                out[batch, s0:s0 + 128] = sh[qb, blk]
    return out


# revision 15
# speedup vs baseline: 1.3534x; 1.0613x over previous
"""Multi-head causal attention (B=4,S=2048,D=768,H=12,HD=64) on 8 Trainium2 cores.

Sharding: 4-way head tensor-parallel (3 heads/core) x 2-way batch data-parallel
(2 batches/core).  Core c: batch group bg=c//4 (batches 2bg,2bg+1), head group
hg=c%4 (heads 3hg..3hg+2).

Per-core device program (SPMD; per-core differences come only from data):
  1. q/k projections emitted transposed (qT,kT: [64 head-dim partitions, rows]);
     v projection row-major with an appended ones column per head (softmax
     denominator rides along the AV matmul as psum row 64).
  2. Causal attention computed transposed: S_T[k,q] = kT.T @ qT, so P=exp(S_T)
     feeds AV directly with no P transpose.  Softmax skips the running max
     (scores are O(1) at this problem's scale; exp is mathematically identical
     to the reference since softmax is shift-invariant).  The two batches of a
     head run concurrently on the PE via 64-row tile packing.  AV accumulates
     ctxU_T[65, q512] = sum_k vE.T @ P_T (row 64 = denominator l).  Normalize:
     1/l via fast-approx DVE reciprocal, broadcast across partitions on GpSimd,
     one fused DVE multiply.
  3. Per 512-row q-block (x2 batches = 1024-row chunk): 8-core AllToAll (bf16,
     128-row shards) redistributes ctx so each core holds all 768 context
     features for its own 2x128 output rows; local projection with full Wp;
     bias via a K=1 ones-outer-product matmul.

Emission is software-pipelined: qk/v projection chunks are interleaved between
attention pairs as PE filler (keeps the PE dense so HAM stays at full clock),
and each chunk's out-projection is emitted one q-block later so the PE never
head-of-line blocks on the AllToAll round trip.

Host side only slices/casts/transposes inputs and concatenates output shards.
"""

import sys

if "/opt/trn_rl_repo" not in sys.path:
    sys.path.insert(0, "/opt/trn_rl_repo")

import numpy as np
import ml_dtypes

BF16 = ml_dtypes.bfloat16

B, S, D = 4, 2048, 768
H, HD = 12, 64
N_CORES = 8
BL = 2          # batches per core
HL = 3          # heads per core
R = BL * S      # 4096 rows per core
KSUB = D // 128  # 6

_CACHE = {}


def _build_nc():
    import concourse.bass as bass  # noqa: F401
    import concourse.tile as tile
    from concourse import bacc, mybir

    f32 = mybir.dt.float32
    bf16 = mybir.dt.bfloat16
    EXP = mybir.ActivationFunctionType.Exp

    nc = bacc.Bacc("TRN2", target_bir_lowering=False, debug=False,
                   num_devices=N_CORES)

    xT_d = nc.dram_tensor("xT", [D, R], bf16, kind="ExternalInput").ap()
    wqk_d = nc.dram_tensor("wqk", [D, 2 * HL * HD], bf16, kind="ExternalInput").ap()
    wv_d = nc.dram_tensor("wv", [D, HL * HD], bf16, kind="ExternalInput").ap()
    wp_d = nc.dram_tensor("wp", [D, D], bf16, kind="ExternalInput").ap()
    bp_d = nc.dram_tensor("bp", [1, D], bf16, kind="ExternalInput").ap()
    mk_d = nc.dram_tensor("mk", [2, 128, 1024], bf16, kind="ExternalInput").ap()
    out_d = nc.dram_tensor("out", [4, 2, 128, D], f32, kind="ExternalOutput").ap()

    RG = [[0, 1, 2, 3, 4, 5, 6, 7]]

    with tile.TileContext(nc) as tc:
        with tc.tile_pool(name="persist", bufs=1) as per, \
             tc.tile_pool(name="dram", bufs=1, space="DRAM") as dram, \
             tc.tile_pool(name="mix_ps", bufs=2, space="PSUM") as mix_ps, \
             tc.tile_pool(name="st_ps", bufs=2, space="PSUM") as st_ps, \
             tc.tile_pool(name="av_ps", bufs=2, space="PSUM") as av_ps, \
             tc.tile_pool(name="pt", bufs=6) as ptp, \
             tc.tile_pool(name="sm", bufs=4) as sm, \
             tc.tile_pool(name="agp", bufs=2) as agp, \
             tc.tile_pool(name="outp", bufs=2) as outp:
            # ---- persistent SBUF tensors -------------------------------
            wqk = per.tile([128, KSUB, 2 * HL * HD], bf16, tag="wqk")
            wv = per.tile([128, KSUB, HL * HD], bf16, tag="wv")
            for j in range(KSUB):
                nc.sync.dma_start(
                    wqk[:, j], wqk_d.rearrange("(o p) c -> p o c", p=128)[:, j])
                nc.sync.dma_start(
                    wv[:, j], wv_d.rearrange("(o p) c -> p o c", p=128)[:, j])
            xT = per.tile([128, KSUB, R], bf16, tag="xT")
            xTr = xT_d.rearrange("(o p) r -> p o r", p=128)
            for rc in range(8):  # per (row-chunk, ksub) pieces
                r0 = rc * 512
                for j in range(KSUB):
                    nc.sync.dma_start(xT[:, j, r0:r0 + 512],
                                      xTr[:, j, r0:r0 + 512])
            masks = per.tile([128, 2, 1024], bf16, tag="mk")
            nc.sync.dma_start(masks[:], mk_d.rearrange("o p c -> p o c"))
            wp = per.tile([128, KSUB, D], bf16, tag="wp")
            nc.sync.dma_start(wp[:], wp_d.rearrange("(o p) c -> p o c", p=128))
            bp_sb = per.tile([1, D], bf16, tag="bp")
            nc.sync.dma_start(bp_sb[:], bp_d[:])
            onesP = per.tile([1, 128], bf16, tag="onesP")
            nc.vector.memset(onesP[:], 1.0)

            # pair p = head p; partitions 0:64 = batch 0, 64:128 = batch 1
            qT = per.tile([128, HL, S], bf16, tag="qT")
            kT = per.tile([128, HL, S], bf16, tag="kT")
            vE = per.tile([128, 2 * 16, HL * 65], bf16, tag="vE")
            nc.vector.memset(vE[:], 1.0)

            warm_in = dram.tile([8 * 192, 8], bf16, tag="warm_in")
            warm_out = dram.tile([8 * 192, 8], bf16, tag="warm_out")
            a2a_in = [dram.tile([8 * 192, 128], bf16, name=f"a2ai{qb}",
                                tag=f"a2ai{qb}") for qb in range(4)]
            a2a_out = [dram.tile([8 * 192, 128], bf16, name=f"a2ao{qb}",
                                 tag=f"a2ao{qb}") for qb in range(4)]

            # ---- emission helpers --------------------------------------
            def emit_qk_ct(b, rc, ct):
                r0 = b * S + rc * 512
                if True:
                    ps = mix_ps.tile([128, 512], f32, tag="mix", name="ps")
                    for j in range(KSUB):
                        nc.tensor.matmul(
                            ps[:],
                            lhsT=wqk[:, j, ct * 128:(ct + 1) * 128],
                            rhs=xT[:, j, r0:r0 + 512],
                            start=(j == 0), stop=(j == KSUB - 1))
                    for half in range(2):
                        gid = 2 * ct + half
                        dest = qT if gid < 3 else kT
                        pair = gid % 3
                        nc.vector.tensor_copy(
                            dest[b * 64:(b + 1) * 64, pair,
                                 rc * 512:(rc + 1) * 512],
                            ps[half * 64:(half + 1) * 64, :])

            def emit_qk_chunk(b, rc):
                for ct in range(3):
                    emit_qk_ct(b, rc, ct)

            def emit_v_tile(b, rt):
                r0 = b * S + rt * 128
                psv = mix_ps.tile([128, HL * HD], f32, tag="mix", name="psv")
                for j in range(KSUB):
                    nc.tensor.matmul(
                        psv[:], lhsT=xT[:, j, r0:r0 + 128], rhs=wv[:, j, :],
                        start=(j == 0), stop=(j == KSUB - 1))
                for h in range(HL):
                    nc.vector.tensor_copy(
                        vE[:, b * 16 + rt, h * 65:h * 65 + 64],
                        psv[:, h * 64:(h + 1) * 64])

            def emit_attn_pair(qb, pair, drain):
                n_k = 4 * (qb + 1)
                n_kp = n_k // 2
                q0 = qb * 512
                avs = [av_ps.tile([65, 512], f32, tag="av", name=f"av{u}")
                       for u in range(2)]
                for kp in range(n_kp):
                    o = kp - (n_kp - 2)  # diag pair offset; >=0 on diagonal
                    qv0 = 256 if o == 1 else 0  # valid q starts here
                    stps = [st_ps.tile([128, 2, 512], f32, tag="st",
                                       name=f"st{u}") for u in range(2)]
                    for i in range(2):
                        for u in range(2):
                            kt = 2 * kp + i
                            nc.tensor.matmul(
                                stps[u][:, i, qv0:512],
                                lhsT=kT[u * 64:(u + 1) * 64, pair,
                                        kt * 128:(kt + 1) * 128],
                                rhs=qT[u * 64:(u + 1) * 64, pair,
                                       q0 + qv0:q0 + 512],
                                start=True, stop=True)
                    for u in range(2):
                        pt = ptp.tile([128, 2, 512], bf16, tag="pt")
                        if qv0:
                            nc.vector.memset(pt[:, :, 0:qv0], 0.0)
                        nc.scalar.activation(pt[:, :, qv0:512],
                                             stps[u][:, :, qv0:512], EXP,
                                             scale=float(HD) ** -0.5)
                        if o >= 0:
                            mk2 = masks[:, o, :].rearrange("p (i c) -> p i c",
                                                           i=2)
                            nc.vector.tensor_mul(pt[:, :, qv0:512],
                                                 pt[:, :, qv0:512],
                                                 mk2[:, :, qv0:512])
                        for i in range(2):
                            kt = 2 * kp + i
                            nc.tensor.matmul(
                                avs[u][:],
                                lhsT=vE[:, u * 16 + kt,
                                        pair * 65:(pair + 1) * 65],
                                rhs=pt[:, i, :],
                                start=(kp == 0 and i == 0),
                                stop=(kp == n_kp - 1 and i == 1))
                    drain(1)
                for u in range(2):
                    lsb = sm.tile([1, 512], f32, tag="lsb")
                    nc.vector.tensor_copy(lsb[:], avs[u][64:65, :])
                    rec = sm.tile([1, 512], f32, tag="rec")
                    nc.vector.reciprocal_approx_fast(rec[:], lsb[:])
                    bcs = sm.tile([64, 512], f32, tag="bcs")
                    nc.gpsimd.partition_broadcast(bcs[:], rec[:])
                    ctxn = sm.tile([64, 512], bf16, tag="ctxn")
                    nc.vector.tensor_mul(ctxn[:], avs[u][0:64, :], bcs[:])
                    for qq in range(4):
                        j = 4 * u + qq
                        nc.sync.dma_start(
                            a2a_in[qb][192 * j + 64 * pair:
                                       192 * j + 64 * (pair + 1), :],
                            ctxn[:, qq * 128:(qq + 1) * 128])

            def emit_a2a(qb):
                nc.gpsimd.collective_compute(
                    "AllToAll", mybir.AluOpType.bypass,
                    ins=[a2a_in[qb][:]], outs=[a2a_out[qb][:]],
                    replica_groups=RG)

            def emit_outproj_blk(qb, blk, ag):
                if blk == 0:
                    nc.sync.dma_start(
                        ag[:], a2a_out[qb].rearrange("(o p) r -> p o r", p=128))
                if True:
                    osb = outp.tile([128, D], f32, tag="osb")
                    for nh in range(2):
                        po = mix_ps.tile([128, 384], f32, tag="mix", name="po")
                        n0 = nh * 384
                        for j in range(KSUB):
                            nc.tensor.matmul(po[:],
                                             lhsT=ag[:, blk * KSUB + j, :],
                                             rhs=wp[:, j, n0:n0 + 384],
                                             start=(j == 0), stop=False)
                        nc.tensor.matmul(po[:], lhsT=onesP[:],
                                         rhs=bp_sb[:, n0:n0 + 384],
                                         start=False, stop=True)
                        nc.vector.tensor_copy(osb[:, n0:n0 + 384], po[:])
                    nc.sync.dma_start(out_d[qb, blk], osb[:])

            # ---- software-pipelined emission ---------------------------
            # warmup collective: absorb ncfw first-call overhead during proj
            nc.sync.dma_start(warm_in[0:128, :], masks[:, 0, 0:8])
            nc.gpsimd.collective_compute(
                "AllToAll", mybir.AluOpType.bypass,
                ins=[warm_in[:]], outs=[warm_out[:]], replica_groups=RG)
            # prologue: everything attention qb0 needs
            emit_qk_chunk(0, 0)
            emit_qk_chunk(1, 0)
            for rt in range(4):
                emit_v_tile(0, rt)
                emit_v_tile(1, rt)

            # filler queue: (need_by_qb, emit_fn); consumed one unit per
            # k-pair inside attention, force-drained at qb boundaries
            from collections import deque
            fq = deque()
            for rc in range(1, 4):
                for b in range(2):
                    for ct in range(3):
                        fq.append((rc, lambda b=b, rc=rc, ct=ct:
                                   emit_qk_ct(b, rc, ct)))
                for b in range(2):
                    for rt in range(4 * rc, 4 * rc + 4):
                        fq.append((rc, lambda b=b, rt=rt: emit_v_tile(b, rt)))

            def drain(n):
                for _ in range(n):
                    if fq:
                        fq.popleft()[1]()

            def drain_needed(qb):
                while fq and fq[0][0] <= qb:
                    fq.popleft()[1]()

            for qb in range(4):
                drain_needed(qb)
                for pair in range(HL):
                    emit_attn_pair(qb, pair, drain)
                    if pair == 1 and qb >= 1:
                        # previous chunk's out-proj; A2A(qb-1) done by now
                        ag = agp.tile([128, 2 * KSUB, 128], bf16, tag="ag",
                                      name=f"ag{qb}")
                        fq.append((9, lambda q=qb - 1, a=ag:
                                   emit_outproj_blk(q, 0, a)))
                        fq.append((9, lambda q=qb - 1, a=ag:
                                   emit_outproj_blk(q, 1, a)))
                emit_a2a(qb)
            drain(99)
            ag3 = agp.tile([128, 2 * KSUB, 128], bf16, tag="ag", name="ag3")
            emit_outproj_blk(3, 0, ag3)
            emit_outproj_blk(3, 1, ag3)

    nc.compile()
    return nc


def _get_nc():
    if "nc" not in _CACHE:
        _CACHE["nc"] = _build_nc()
    return _CACHE["nc"]


def _masks_np():
    k = np.arange(128)[:, None]
    q = np.arange(512)[None, :]
    tiles = [(q >= k + 128 * t) for t in range(4)]
    m = np.stack([np.concatenate([tiles[2 * o], tiles[2 * o + 1]], axis=1)
                  for o in range(2)])
    return m.astype(BF16)


def _prep_in_maps(x, Wq, Wk, Wv, Wp, bp):
    x = np.asarray(x, dtype=np.float32)
    mk = _masks_np()
    wp_full = np.asarray(Wp).astype(BF16)
    bp_row = np.asarray(bp, dtype=np.float32).reshape(1, D).astype(BF16)
    xT_bg = []
    for bg in range(2):
        xl = x[2 * bg:2 * bg + 2].reshape(R, D)
        xT_bg.append(np.ascontiguousarray(xl.T).astype(BF16))
    wqk_hg, wv_hg = [], []
    for hg in range(4):
        hs = slice(192 * hg, 192 * (hg + 1))
        wqk_hg.append(np.concatenate(
            [np.asarray(Wq)[:, hs], np.asarray(Wk)[:, hs]], axis=1).astype(BF16))
        wv_hg.append(np.asarray(Wv)[:, hs].astype(BF16))
    in_maps = []
    for c in range(N_CORES):
        bg, hg = c // 4, c % 4
        in_maps.append({
            "xT": xT_bg[bg],
            "wqk": wqk_hg[hg],
            "wv": wv_hg[hg],
            "wp": wp_full,
            "bp": bp_row,
            "mk": mk,
        })
    return in_maps


def kernel(x, Wq, Wk, Wv, Wp, bp):
    from concourse import bass_utils

    nc = _get_nc()
    in_maps = _prep_in_maps(x, Wq, Wk, Wv, Wp, bp)
    res = bass_utils.run_bass_kernel_spmd(nc, in_maps,
                                          core_ids=list(range(N_CORES)))
    out = np.empty((B, S, D), np.float32)
    for c in range(N_CORES):
        sh = res.results[c]["out"]  # [4 chunks, 2 blocks, 128, D]
        for qb in range(4):
            for blk in range(2):
                batch = 2 * blk + c // 4
                s0 = 512 * qb + 128 * (c % 4)
                out[batch, s0:s0 + 128] = sh[qb, blk]
    return out
